# revision 36
# baseline (speedup 1.0000x reference)
"""Trainium2 Bass kernel for CausalWanSelfAttention (KV-cache-bias attention).

Math: disjoint-segment attention + LSE merge == one global softmax with a
per-key bias b_l (log 0.1 on keys in [frame_seqlen, current_block_start)).
exp needs no max-subtraction; out = (E @ V) / (1^T E) with
E = exp(scale*S + b_l), the bias applied per PSUM partition (= key) via
the activation instruction's per-partition bias operand.

Sharding: 24 units = (head, q-half) pairs, 3 per core; each unit is 1024
queries x 8192 keys, processed as 64 key-chunks of 128.

v12 (from v9's 231us / 225us): chunk-granular pipeline. Q is host-prescaled
by 128*scale/ln2 (the ACT free-affine compensates with scale=ln2/128), so
any engine could consume S in log2-bit units. One chunk-slot = S' tile
[128k, 1024q] (2 PSUM banks, s_pool bufs=3 -> A runs up to 3 slots ahead
of exp, hiding sem latency; this -3 slot slack is what v9's 2-deep group
rotation lacked). exp = ONE ACT instruction per chunk (N=1024, per-key AP
bias works for segment-boundary-straddling chunks too, so no fuse-mask
rebuilds: a single program serves every (frame_seqlen, block_start)).
B lags 2 slots. Norm tree ops span both q-halves ([128,1024] bf16
tensor_tensor at 2x), halving DVE op count vs v9; quad sums + last-4 raw
E chunks are DMA'd out and the host does the final 128-partition sum (it
already divides).

Measured spread of engine busy (timeline sim + HW calibration): ACT ~220us
(192 exp instrs x (1024+352)cyc/1.2GHz -- the critical path), PE ~164us
(768 MM N=512, ldweights hidden), DVE ~84us, DMA ~100us. HW ~222-226us.
Explored and rejected (all measured slower on HW): exp offload to DVE via
a custom 8-slice exp2-bits op (bit-exact on HW, 0.47% max rel err, but
DVE-from-PSUM streams ~1.8x slower than ACT and mixing never beat pure
ACT); N=1536 exp instrs (PSUM depth drops to 2, costs more in pipeline
coupling than the 100ns/chunk instruction-overhead saving); Schraudolph
on DVE; batched-B decoupling; GPSIMD/DMA exp routes (no PSUM access).
"""

import math
import sys

for _p in ("/opt/trn_rl_repo",):
    if _p not in sys.path:
        sys.path.insert(0, _p)

import numpy as np
import ml_dtypes

import concourse.bass as bass
import concourse.mybir as mybir
import concourse.tile as tile
from concourse import bacc
from concourse.bass_utils import run_bass_kernel_spmd

BF16 = mybir.dt.bfloat16
F32 = mybir.dt.float32
NP_BF16 = ml_dtypes.bfloat16

B, LQ, LK, H, D = 1, 2048, 8192, 12, 128
N_CORES = 8
UNITS_PER_CORE = 3          # 24 units = 12 heads x 2 q-halves
QSPAN = 1024                # queries per unit
HS = 512
NLC = LK // 128             # 64 key chunks of 128
NCT = UNITS_PER_CORE * NLC  # 192 chunk-slots per core
NQUAD = NLC // 4            # 16 norm slots (last = 4 raw chunks)
SCALE = 1.0 / math.sqrt(D)
LN2 = math.log(2.0)
LOG_BIAS = math.log(0.1)
QPRE = 128.0 * SCALE / LN2          # host pre-scale on q
ACT_SCALE = LN2 / 128.0             # ACT exp free-affine compensation

LAG = 2           # B lags A by this many chunk-slots

_CACHED = {}
TIME_LOOP = 1     # timing experiments only: hardware-loop the body N times


def _build_program():
    nc = bacc.Bacc("TRN2", target_bir_lowering=False, debug=False,
                   enable_asserts=False)

    qt_d = nc.dram_tensor("qt", [UNITS_PER_CORE, 128, QSPAN], BF16,
                          kind="ExternalInput")
    kt_d = nc.dram_tensor("kt", [UNITS_PER_CORE, 128, LK], BF16,
                          kind="ExternalInput")
    vl_d = nc.dram_tensor("vl", [UNITS_PER_CORE, LK, 128], BF16,
                          kind="ExternalInput")
    bias_d = nc.dram_tensor("bias", [128, NLC], F32, kind="ExternalInput")
    ot_d = nc.dram_tensor("ot", [UNITS_PER_CORE, 128, QSPAN], F32,
                          kind="ExternalOutput")
    no_d = nc.dram_tensor("no", [UNITS_PER_CORE, NQUAD + 3, 128, QSPAN], BF16,
                          kind="ExternalOutput")

    qt_ap = qt_d.ap()
    kt_ap = kt_d.ap()
    # [u, (c p), d] -> [u, p, c, d]: partition = key index within chunk
    vl_ap = vl_d.ap().rearrange("u (c p) d -> u p c d", p=128)
    bias_ap = bias_d.ap()
    ot_ap = ot_d.ap()
    no_ap = no_d.ap()

    with tile.TileContext(nc) as tc:
        with (
            tc.tile_pool(name="kt_pool", bufs=2) as kt_pool,
            tc.tile_pool(name="vl_pool", bufs=2) as vl_pool,
            tc.tile_pool(name="qt_pool", bufs=2) as qt_pool,
            tc.tile_pool(name="cn_pool", bufs=1) as cn_pool,
            tc.tile_pool(name="e_pool", bufs=LAG + 4) as e_pool,
            tc.tile_pool(name="pp_pool", bufs=3) as pp_pool,
            tc.tile_pool(name="qq_pool", bufs=3) as qq_pool,
            tc.tile_pool(name="ob_pool", bufs=2) as ob_pool,
            tc.tile_pool(name="s_pool", bufs=3, space="PSUM") as s_pool,
            tc.tile_pool(name="o_pool", bufs=1, space="PSUM") as o_pool,
        ):
            bias_t = cn_pool.tile([128, NLC], F32, name="bias_t")
            # Warmup: exp table-set load overlaps the first input DMA
            warm_t = cn_pool.tile([128, 1], F32, name="warm_t")
            nc.vector.memset(warm_t[:], 0.0)
            nc.scalar.activation(warm_t[:], warm_t[:],
                                 mybir.ActivationFunctionType.Exp)

            loaded, cur = {}, {}

            def start_load(u):
                qt = qt_pool.tile([128, QSPAN], BF16, name=f"qt_u{u}",
                                  tag="qt")
                nc.sync.dma_start(out=qt[:], in_=qt_ap[u])
                kt = kt_pool.tile([128, LK], BF16, name=f"kt_u{u}", tag="kt")
                vl = vl_pool.tile([128, NLC, 128], BF16,
                                  name=f"vl_u{u}", tag="vl")
                loaded[u] = (kt, vl, qt)

            def load_slice(u, idx, den):
                kt, vl, qt = loaded.get(u) or cur[u]
                slk = bass.ts(idx, LK // den)
                nc.sync.dma_start(out=kt[:, slk], in_=kt_ap[u][:, slk])
                slv = bass.ts(idx, NLC // den)
                nc.sync.dma_start(out=vl[:, slv, :], in_=vl_ap[u][:, slv, :])

            # unit 0 lead-in: first chunks' deps first, then the rest
            qt0 = qt_pool.tile([128, QSPAN], BF16, name="qt_u0", tag="qt")
            kt0 = kt_pool.tile([128, LK], BF16, name="kt_u0", tag="kt")
            vl0 = vl_pool.tile([128, NLC, 128], BF16, name="vl_u0", tag="vl")
            loaded[0] = (kt0, vl0, qt0)
            nc.sync.dma_start(out=kt0[:, 0:256], in_=kt_ap[0][:, 0:256])
            nc.sync.dma_start(out=qt0[:], in_=qt_ap[0])
            nc.sync.dma_start(out=bias_t[:], in_=bias_ap)
            nc.sync.dma_start(out=vl0[:, 0:4, :], in_=vl_ap[0][:, 0:4, :])
            nc.sync.dma_start(out=kt0[:, 256:512], in_=kt_ap[0][:, 256:512])
            for idx in range(1, 16):
                slk = bass.ts(idx, LK // 16)
                nc.sync.dma_start(out=kt0[:, slk], in_=kt_ap[0][:, slk])
                slv = bass.ts(idx, NLC // 16)
                nc.sync.dma_start(out=vl0[:, slv, :], in_=vl_ap[0][:, slv, :])

            import contextlib
            loop_cm = (tc.For_i(0, TIME_LOOP, 1) if TIME_LOOP > 1
                       else contextlib.nullcontext())
            loop_cm.__enter__()

            ot_t = {}
            echunk = {}         # (unit, chunk) -> e tile
            pt = {}             # (unit, pair) -> pp tile
            for g in range(NCT + LAG + 1):
                if g < NCT:
                    ug, c = g // NLC, g % NLC
                    if c == 0:
                        cur[ug] = loaded.pop(ug)
                    kt, vl, qt = cur[ug]
                    sg = s_pool.tile([128, QSPAN], F32, tag="s", name=f"s_{g}")
                    for qh in range(2):
                        nc.tensor.matmul(
                            sg[:, bass.ts(qh, HS)],
                            lhsT=kt[:, bass.ts(c, 128)],
                            rhs=qt[:, bass.ts(qh, HS)],
                            start=True, stop=True)
                    e = e_pool.tile([128, QSPAN], BF16, tag="e", name=f"e_{g}")
                    nc.scalar.activation(
                        e[:], sg[:],
                        mybir.ActivationFunctionType.Exp,
                        bias=bias_t[:, c:c + 1],
                        scale=ACT_SCALE)
                    echunk[(ug, c)] = e
                    # next unit's inputs, spread (one eighth per 4 slots)
                    if ug + 1 < UNITS_PER_CORE:
                        if c == 12:
                            start_load(ug + 1)
                        elif c >= 16 and c < 48 and c % 4 == 0:
                            load_slice(ug + 1, c // 4 - 4, 8)
                d = g - LAG
                if 0 <= d < NCT:
                    ud, c = d // NLC, d % NLC
                    if c == 0:
                        ot_t[ud] = o_pool.tile([128, QSPAN], F32,
                                               name=f"ot_u{ud}", tag="ot")
                    e = echunk[(ud, c)]
                    for qh in range(2):
                        nc.tensor.matmul(
                            ot_t[ud][:, bass.ts(qh, HS)],
                            lhsT=cur[ud][1][:, c, :],
                            rhs=e[:, bass.ts(qh, HS)],
                            start=(c == 0), stop=(c == NLC - 1))
                    if c >= NLC - 4:
                        # unit tail: raw E chunks, slots 15..18 (must stay
                        # here after B: moving this DMA up to emit_a right
                        # after the exp measured rel err 6e-2 on HW)
                        nc.sync.dma_start(
                            out=no_ap[ud][NQUAD - 1 + c - (NLC - 4)],
                            in_=e[:])
                    elif c % 2 == 1:
                        pp = pp_pool.tile([128, QSPAN], BF16, tag="pp",
                                          name=f"pp_{d}")
                        nc.vector.tensor_add(
                            pp[:], echunk[(ud, c - 1)][:], e[:])
                        pt[(ud, c // 2)] = pp
                        if c % 4 == 3:
                            qq = qq_pool.tile([128, QSPAN], BF16, tag="qq",
                                              name=f"qq_{d}")
                            nc.vector.tensor_add(
                                qq[:], pt.pop((ud, c // 2 - 1))[:],
                                pt.pop((ud, c // 2))[:])
                            nc.sync.dma_start(out=no_ap[ud][c // 4], in_=qq[:])
                    if c == NLC - 1:
                        for cc in range(NLC):
                            echunk.pop((ud, cc), None)
                        ot = ot_t.pop(ud)
                        ot_sb = ob_pool.tile([128, QSPAN], F32,
                                             name=f"otsb_u{ud}", tag="otsb")
                        last = ud == UNITS_PER_CORE - 1
                        if last:
                            # tail: ACT copies + issues the DMA so the final
                            # drain overlaps the last exp/norm work
                            for half in range(2):
                                sl = bass.ts(half, HS)
                                nc.scalar.copy(ot_sb[:, sl], ot[:, sl])
                                nc.scalar.dma_start(out=ot_ap[ud][:, sl],
                                                    in_=ot_sb[:, sl])
                        else:
                            # one full-tile copy: frees the O PSUM banks for
                            # the next unit with half the instruction overhead
                            nc.vector.tensor_scalar_add(ot_sb[:], ot[:], 0.0)
                            nc.sync.dma_start(out=ot_ap[ud], in_=ot_sb[:])

            loop_cm.__exit__(None, None, None)

    nc.compile()
    return nc


def _get_program(key=None):
    # One program serves every (frame_seqlen, current_block_start): the
    # per-key bias is runtime data (bias tensor), applied per partition.
    if "prog" not in _CACHED:
        _CACHED["prog"] = _build_program()
    return _CACHED["prog"]


def _host_prep(q, k, v, frame_seqlen, current_block_start):
    fs = max(0, min(int(frame_seqlen), LK))
    bs = max(0, min(int(current_block_start), LK))
    logw = np.zeros(LK, np.float32)
    if bs >= fs:
        logw[fs:bs] = LOG_BIAS
    else:
        # reference skips the empty middle segment and its outer segments
        # [0, fs) and [bs, LK) overlap on [bs, fs): those keys count twice
        logw[bs:fs] = math.log(2.0)
    bias = np.ascontiguousarray(logw.reshape(NLC, 128).T)  # [128, NLC]

    q = np.asarray(q, dtype=np.float32)
    k = np.asarray(k, dtype=np.float32)
    v = np.asarray(v, dtype=np.float32)

    qT = np.ascontiguousarray(
        (q[0] * QPRE).transpose(1, 2, 0)).astype(NP_BF16)   # [H,128,LQ]
    kT = np.ascontiguousarray(k[0].transpose(1, 2, 0)).astype(NP_BF16)
    vL = np.ascontiguousarray(v[0].transpose(1, 0, 2)).astype(NP_BF16)

    in_maps = []
    for i in range(N_CORES):
        units = [3 * i + uu for uu in range(UNITS_PER_CORE)]
        heads = [g // 2 for g in units]
        qhs = [g % 2 for g in units]
        in_maps.append({
            "qt": np.ascontiguousarray(
                np.stack([qT[h, :, qh * QSPAN:(qh + 1) * QSPAN]
                          for h, qh in zip(heads, qhs)])),
            "kt": np.ascontiguousarray(np.stack([kT[h] for h in heads])),
            "vl": np.ascontiguousarray(np.stack([vL[h] for h in heads])),
            "bias": bias,
        })
    return in_maps, None


def _assemble(results):
    out = np.empty((B, LQ, H, D), np.float32)
    for i in range(N_CORES):
        ot = results[i]["ot"]   # [3, 128, 1024] unnormalized O^T
        nm = results[i]["no"].astype(np.float32).sum(axis=(1, 2))  # [3, 1024]
        for uu in range(UNITS_PER_CORE):
            g = 3 * i + uu
            h, qh = g // 2, g % 2
            out[0, qh * QSPAN:(qh + 1) * QSPAN, h, :] = (
                ot[uu] / nm[uu][None, :]).T
    return out


def kernel(q, k, v, frame_seqlen, current_block_start):
    in_maps, key = _host_prep(q, k, v, frame_seqlen, current_block_start)
    nc = _get_program(key)
    res = run_bass_kernel_spmd(nc, in_maps, core_ids=list(range(N_CORES)))
    return _assemble(res.results)
